# revision 11
# baseline (speedup 1.0000x reference)
"""Self-contained Trainium2 Bass kernel for nn_BaseNeighborlist (pair cutoff screening).

kernel(coordinates, neighborlist) -> (screened_diff, screened_dist, in_cutoff)

Sharding: the 4M-pair dimension is split evenly across 8 NeuronCores, with the
small coordinate table replicated to every core (data parallel over pairs).
On device, each core gathers both endpoints' coordinates per pair using the
SWDGE dma_gather engine (one atom per 256B row: 64 fp32, xyz in the first 3),
computes diff / dist / cutoff mask on DVE+ACT, and streams results out.

Pair j of core c sits at (tile t, partition p, column j') with
j = t*8192 + p*64 + j'; dma_gather position i = 128*j' + p.
Raw bacc with manual semaphores (Tile cannot schedule async SWDGE gathers).
"""

import numpy as np

N_CORES = 8
N_ATOMS = 16384
N_PAIRS = 4_194_304
PAIRS_PER_CORE = N_PAIRS // N_CORES   # 524288
CUTOFF = np.float32(5.2)

J = 64                  # pairs per partition per tile
NT = 128 * J            # pairs per tile (= num_idxs per dma_gather, carveout-limited to 8192)
S = NT // 16            # wrapped idx columns
T = PAIRS_PER_CORE // NT  # tiles per core (64)
ELEM = 64               # fp32 elems per gathered row (256B)


def _d2_threshold() -> float:
    """Largest fp32 t with sqrt_rn(t) <= CUTOFF, so (d2 <= t) == (sqrt(d2) <= CUTOFF)."""
    t = np.float32(CUTOFF) * np.float32(CUTOFF)
    while np.sqrt(np.nextafter(t, np.float32(np.inf), dtype=np.float32)) <= CUTOFF:
        t = np.nextafter(t, np.float32(np.inf), dtype=np.float32)
    while np.sqrt(t) > CUTOFF:
        t = np.nextafter(t, np.float32(-np.inf), dtype=np.float32)
    return float(t)


def build(n_tiles: int = T, reps: int = 1):
    import concourse.bacc as bacc
    import concourse.bass as bass
    import concourse.mybir as mybir
    from concourse import library_config

    nt = n_tiles
    f32 = mybir.dt.float32
    i16 = mybir.dt.int16
    u8 = mybir.dt.uint8
    T_D2 = _d2_threshold()
    Alu = mybir.AluOpType

    nc = bacc.Bacc("TRN2", target_bir_lowering=False)
    table = nc.dram_tensor("table", [N_ATOMS, ELEM], f32, kind="ExternalInput")
    idx0 = nc.dram_tensor("idx0", [nt, 128, S], i16, kind="ExternalInput")
    idx1 = nc.dram_tensor("idx1", [nt, 128, S], i16, kind="ExternalInput")
    sdiff = nc.dram_tensor("sdiff", [nt, 128, 3 * J], f32, kind="ExternalOutput")
    sdist = nc.dram_tensor("sdist", [nt, 128, J], f32, kind="ExternalOutput")
    mask8 = nc.dram_tensor("mask8", [nt, 128, J], u8, kind="ExternalOutput")

    it0 = nc.alloc_sbuf_tensor("it0", [128, S], i16)
    it1 = nc.alloc_sbuf_tensor("it1", [128, S], i16)
    g0 = nc.alloc_sbuf_tensor("g0", [128, J * ELEM], f32)
    g1 = nc.alloc_sbuf_tensor("g1", [128, J * ELEM], f32)
    diff = nc.alloc_sbuf_tensor("diff", [128, 3 * J], f32)
    sq = nc.alloc_sbuf_tensor("sq", [128, 3 * J], f32)
    sdf = nc.alloc_sbuf_tensor("sdf", [128, 3 * J], f32)
    d2 = nc.alloc_sbuf_tensor("d2", [128, J], f32)
    dist = nc.alloc_sbuf_tensor("dist", [128, J], f32)
    maskf = nc.alloc_sbuf_tensor("maskf", [128, J], f32)
    sd = nc.alloc_sbuf_tensor("sd", [128, J], f32)
    m8 = nc.alloc_sbuf_tensor("m8", [128, J], u8)

    s_io = nc.alloc_semaphore("s_io")    # idx input DMAs (16/dma)
    NS = 4
    s_gs = [nc.alloc_semaphore(f"s_g{i}") for i in range(NS)]  # rotating gather sems
    s_v1 = nc.alloc_semaphore("s_v1")    # vector stage 1 (diff/sq/d2) done
    s_a = nc.alloc_semaphore("s_a")      # sqrt done
    s_v2 = nc.alloc_semaphore("s_v2")    # vector stage 2 (outputs ready)
    s_od = nc.alloc_semaphore("s_od")    # output DMAs (3x16/tile)
    s_dv = nc.alloc_semaphore("s_dv")    # intra-DVE RAW chain

    g0v = g0[:].rearrange("p (j e) -> p j e", e=ELEM)[:, :, :3]
    g1v = g1[:].rearrange("p (j e) -> p j e", e=ELEM)[:, :, :3]
    diffv = diff[:].rearrange("p (j c) -> p j c", c=3)
    sqv = sq[:].rearrange("p (j c) -> p j c", c=3)
    sdfv = sdf[:].rearrange("p (j c) -> p j c", c=3)
    maskb = maskf[:].unsqueeze(2).to_broadcast((128, J, 3))

    R = range(reps * nt)
    tt = lambda k: k % nt  # dram tile index for step k

    with nc.Block() as block:

        @block.sync
        def _(sync: bass.BassEngine):
            for k in R:
                if k > 0:
                    # outputs of step k-1 (also frees sdf/sd/m8 for vector WAR)
                    sync.wait_ge(s_v2, k)
                    sync.dma_start(sdiff[tt(k - 1)], sdf[:]).then_inc(s_od, 16)
                    sync.dma_start(sdist[tt(k - 1)], sd[:]).then_inc(s_od, 16)
                    sync.dma_start(mask8[tt(k - 1)], m8[:]).then_inc(s_od, 16)
                # inputs of step k (idx bufs freed once gathers k-1 completed)
                if k > 0:
                    sync.wait_ge(s_gs[(k - 1) % NS], 32 * ((k - 1) // NS + 1))
                sync.dma_start(it0[:], idx0[tt(k)]).then_inc(s_io, 16)
                sync.dma_start(it1[:], idx1[tt(k)]).then_inc(s_io, 16)
            sync.wait_ge(s_v2, reps * nt)
            sync.dma_start(sdiff[tt(reps * nt - 1)], sdf[:]).then_inc(s_od, 16)
            sync.dma_start(sdist[tt(reps * nt - 1)], sd[:]).then_inc(s_od, 16)
            sync.dma_start(mask8[tt(reps * nt - 1)], m8[:]).then_inc(s_od, 16)
            sync.wait_ge(s_od, 48 * reps * nt)

        @block.gpsimd
        def _(gpsimd: bass.BassGpSimd):
            gpsimd.load_library(library_config.mlp)
            for k in R:
                gpsimd.wait_ge(s_io, 32 * (k + 1))
                if k > 0:
                    gpsimd.wait_ge(s_v1, k)  # g0/g1 free (diff of k-1 done)
                gpsimd.dma_gather(
                    g0[:].rearrange("p (j e) -> p j e", e=ELEM), table[:], it0[:],
                    NT, NT, ELEM, single_packet=False,
                ).then_inc(s_gs[k % NS], 16)
                gpsimd.dma_gather(
                    g1[:].rearrange("p (j e) -> p j e", e=ELEM), table[:], it1[:],
                    NT, NT, ELEM, single_packet=False,
                ).then_inc(s_gs[k % NS], 16)

        @block.vector
        def _(vector):
            for k in R:
                b = 3 * k  # intra-DVE chain counter base
                vector.wait_ge(s_gs[k % NS], 32 * (k // NS + 1))
                nc.vector.tensor_tensor(out=diffv, in0=g0v, in1=g1v,
                                        op=Alu.subtract).then_inc(s_dv, 1)
                nc.vector.wait_ge(s_dv, b + 1)
                nc.vector.tensor_tensor(out=sq[:], in0=diff[:], in1=diff[:],
                                        op=Alu.mult).then_inc(s_dv, 1)
                nc.vector.wait_ge(s_dv, b + 2)
                nc.vector.tensor_reduce(
                    out=d2[:], in_=sqv, axis=mybir.AxisListType.X, op=Alu.add,
                ).then_inc(s_v1, 1)
                nc.vector.wait_ge(s_v1, k + 1)
                nc.vector.wait_ge(s_a, k + 1)
                if k > 0:
                    nc.vector.wait_ge(s_od, 48 * k)  # sdf/sd/m8 free
                nc.vector.tensor_scalar(out=maskf[:], in0=d2[:], scalar1=T_D2,
                                        scalar2=None, op0=Alu.is_le).then_inc(s_dv, 1)
                nc.vector.tensor_scalar(out=m8[:], in0=d2[:], scalar1=T_D2,
                                        scalar2=None, op0=Alu.is_le)
                nc.vector.wait_ge(s_dv, b + 3)
                nc.vector.tensor_tensor(out=sd[:], in0=dist[:], in1=maskf[:], op=Alu.mult)
                nc.vector.tensor_tensor(out=sdfv, in0=diffv, in1=maskb,
                                        op=Alu.mult).then_inc(s_v2, 1)

        @block.scalar
        def _(scalar):
            for k in R:
                scalar.wait_ge(s_v1, k + 1)
                nc.scalar.activation(
                    out=dist[:], in_=d2[:], func=mybir.ActivationFunctionType.Sqrt,
                ).then_inc(s_a, 1)

    nc.compile()
    return nc


_cached_nc = None


def _get_nc():
    global _cached_nc
    if _cached_nc is None:
        _cached_nc = build()
    return _cached_nc


def prep_idx(flat_idx: np.ndarray, nt: int) -> np.ndarray:
    """Pack a flat per-core pair-endpoint index list into the wrapped,
    replicated [nt, 128, S] int16 layout dma_gather consumes."""
    a = np.ascontiguousarray(flat_idx, dtype=np.int16).reshape(nt, 128, J)
    # position-major list: pos i = 128*j' + p holds pair (p, j')
    lst = a.transpose(0, 2, 1).reshape(nt, NT)
    # wrap into 16 partitions, replicate to 128
    w16 = lst.reshape(nt, S, 16).transpose(0, 2, 1)
    return np.broadcast_to(w16[:, None], (nt, 8, 16, S)).reshape(nt, 128, S).copy()


def make_table(coordinates: np.ndarray) -> np.ndarray:
    tab = np.zeros((N_ATOMS, ELEM), dtype=np.float32)
    tab[:, :3] = np.ascontiguousarray(coordinates, dtype=np.float32).reshape(N_ATOMS, 3)
    return tab


def make_in_maps(coordinates: np.ndarray, neighborlist: np.ndarray):
    tab = make_table(coordinates)
    nl = np.asarray(neighborlist)
    return [
        {
            "table": tab,
            "idx0": prep_idx(nl[0, c * PAIRS_PER_CORE:(c + 1) * PAIRS_PER_CORE], T),
            "idx1": prep_idx(nl[1, c * PAIRS_PER_CORE:(c + 1) * PAIRS_PER_CORE], T),
        }
        for c in range(N_CORES)
    ]


def assemble(results):
    sdiff = np.concatenate(
        [r["sdiff"].reshape(PAIRS_PER_CORE, 3) for r in results], axis=0
    )
    sdist = np.concatenate(
        [r["sdist"].reshape(PAIRS_PER_CORE) for r in results], axis=0
    )
    mask = np.concatenate(
        [r["mask8"].reshape(PAIRS_PER_CORE) for r in results], axis=0
    ).astype(bool)
    return sdiff, sdist, mask


def kernel(coordinates: np.ndarray, neighborlist: np.ndarray):
    from concourse.bass_utils import run_bass_kernel_spmd

    nc = _get_nc()
    in_maps = make_in_maps(coordinates, neighborlist)
    res = run_bass_kernel_spmd(nc, in_maps, list(range(N_CORES)))
    return assemble(res.results)
